# revision 15
# baseline (speedup 1.0000x reference)
"""DeepSeek-MoE layer on 8 Trainium2 NeuronCores.

Strategy: data-parallel over tokens (512 tokens/core, all weights replicated).
Each core computes the router, the shared SwiGLU expert and all 8 routed
experts (dense grouped GEMM, matching the reference training path) for its
token slice in a transposed layout: features on SBUF partitions, tokens on
the free dimension.

v2 over the first working version:
- All big matmuls run in bf16 (weights cast host-side, x cast host-side):
  same PE rate as f32r but half the HBM traffic and half the LDWEIGHTS
  time, which was barely hidden behind 512-row matmuls.
- The router (true-fp32 scores for exact top-2) is emitted FIRST so its
  serial DVE/ACT selection chain runs under the shared-expert matmul
  stream instead of stalling the in-order PE mid-kernel.
- Gate application is decoupled from routed stage-1: H = gelu(rw1 @ x) is
  computed for all experts with no dependence on the router output; the
  comb broadcasts + elementwise gate multiplies happen afterwards, so no
  PE matmul ever waits on the router chain.

No collectives: the host concatenates the 8 per-core [1024, 512] output
slices (transposed back) into the full [2, 2048, 1024] output.
"""

import sys

sys.path.insert(0, "/opt/trn_rl_repo")

import numpy as np
import ml_dtypes

import concourse.bass as bass
import concourse.bacc as bacc
import concourse.mybir as mybir
import concourse.tile as tile
from concourse.bass_utils import run_bass_kernel_spmd
from concourse.masks import make_identity

F32 = mybir.dt.float32
BF16 = mybir.dt.bfloat16
AF = mybir.ActivationFunctionType
ALU = mybir.AluOpType
AX = mybir.AxisListType

P = 128          # partitions
NCORES = 8
B, T, D = 2, 2048, 1024
N = B * T        # 4096 tokens
TOK = N // NCORES  # 512 tokens per core
HS = 2048        # shared expert hidden
HR = 512         # routed expert hidden
E = 8            # experts
KD = D // P      # 8  k-tiles over d
NHS = HS // P    # 16 h_s tiles
NHR = HR // P    # 4  h_r tiles
ND = D // P      # 8  output d tiles
TOP_K = 2
EPS = 1e-9
OUT_SCALE = 1.0 / 3.0  # 1 / (N_SHARED + TOP_K)

HC = 256               # h-chunk (columns of sw1/sw3 loaded per DMA)
NHC = HS // HC         # 8 chunks
HTPC = HC // P         # 2 h-tiles per chunk

NB = np.bfloat16 if hasattr(np, "bfloat16") else ml_dtypes.bfloat16


def _emit(nc, tc, reps=1):
    xT = nc.dram_tensor("xT", [D, TOK], BF16, kind="ExternalInput")
    xTf = nc.dram_tensor("xTf", [D, TOK], F32, kind="ExternalInput")
    tembT = nc.dram_tensor("tembT", [D, 1], F32, kind="ExternalInput")
    rwT = nc.dram_tensor("rwT", [D, E], F32, kind="ExternalInput")
    rtwT = nc.dram_tensor("rtwT", [D, E], F32, kind="ExternalInput")
    biasB = nc.dram_tensor("biasB", [P, E], F32, kind="ExternalInput")
    sw1T = nc.dram_tensor("sw1T", [D, HS], BF16, kind="ExternalInput")
    sw3T = nc.dram_tensor("sw3T", [D, HS], BF16, kind="ExternalInput")
    sw2T = nc.dram_tensor("sw2T", [HS, D], BF16, kind="ExternalInput")
    rw1T = nc.dram_tensor("rw1T", [E, D, HR], BF16, kind="ExternalInput")
    rw2T = nc.dram_tensor("rw2T", [E, HR, D], BF16, kind="ExternalInput")
    outT = nc.dram_tensor("outT", [D, TOK], F32, kind="ExternalOutput")

    # DRAM views with 128-partition tiling
    xT_v = xT[:].rearrange("(k p) t -> p k t", p=P)            # [128, 8, 512]
    xTf_v = xTf[:].rearrange("(k p) t -> p k t", p=P)
    tembT_v = tembT[:].rearrange("(k p) o -> p k o", p=P)      # [128, 8, 1]
    rwT_v = rwT[:].rearrange("(k p) e -> p k e", p=P)          # [128, 8, 8]
    rtwT_v = rtwT[:].rearrange("(k p) e -> p k e", p=P)
    sw1T_v = sw1T[:].rearrange("(k p) h -> p k h", p=P)        # [128, 8, 2048]
    sw3T_v = sw3T[:].rearrange("(k p) h -> p k h", p=P)
    sw2T_v = sw2T[:].rearrange("(k p) d -> p k d", p=P)        # [128, 16, 1024]
    rw1T_v = rw1T[:].rearrange("e (k p) h -> p e k h", p=P)    # [128, 8, 8, 512]
    rw2T_v = rw2T[:].rearrange("e (k p) d -> p e k d", p=P)    # [128, 8, 4, 1024]
    outT_v = outT[:].rearrange("(dt p) t -> dt p t", p=P)      # [8, 128, 512]

    with (
        tc.tile_pool(name="pconst", bufs=1) as pconst,
        tc.tile_pool(name="pact", bufs=1) as pact,
        tc.tile_pool(name="pstream", bufs=6) as pstream,
        tc.tile_pool(name="pw2", bufs=3) as pw2,
        tc.tile_pool(name="pcb", bufs=8) as pcb,
        tc.tile_pool(name="ptmp", bufs=2) as ptmp,
        tc.tile_pool(name="pout", bufs=2) as pout,
        tc.tile_pool(name="prt", bufs=1) as prt,
        tc.tile_pool(name="ps", bufs=8, space="PSUM") as ps,
    ):
      for _rep in range(reps):
        # ---- resident inputs ----
        # DMA issue order is tuned for the startup window: the PE's first
        # work is shared chunk 0, so its weights and the bf16 x tiles come
        # first; the router's fp32 x tiles stream under chunk 0's compute.
        preload = {}
        for hc in (0, 1):
            w1c = pstream.tile([P, KD, HC], BF16, tag="wstream")
            nc.sync.dma_start(w1c[:], sw1T_v[:, :, hc * HC:(hc + 1) * HC])
            w3c = pstream.tile([P, KD, HC], BF16, tag="wstream")
            nc.sync.dma_start(w3c[:], sw3T_v[:, :, hc * HC:(hc + 1) * HC])
            preload[hc] = (w1c, w3c)
        xk = []
        for k in range(KD):
            t = pconst.tile([P, TOK], BF16, tag=f"xt{k}")
            nc.sync.dma_start(t[:], xT_v[:, k, :])
            xk.append(t)
        rwt = pconst.tile([P, KD, E], F32, tag="rwt")
        nc.sync.dma_start(rwt[:], rwT_v)
        tembt = pconst.tile([P, KD, 1], F32, tag="tembt")
        nc.sync.dma_start(tembt[:], tembT_v)
        rtwt = pconst.tile([P, KD, E], F32, tag="rtwt")
        nc.sync.dma_start(rtwt[:], rtwT_v)
        xf = []
        for k in range(KD):
            t = pconst.tile([P, TOK], F32, tag=f"xf{k}")
            nc.sync.dma_start(t[:], xTf_v[:, k, :])
            xf.append(t)
        biasb = pconst.tile([P, E], F32, tag="biasb")
        nc.sync.dma_start(biasb[:], biasB[:])
        ident = pconst.tile([P, P], F32, tag="ident")
        make_identity(nc, ident[:])
        ones1 = pconst.tile([1, P], BF16, tag="ones1")
        nc.vector.memset(ones1[:], 1.0)

        # ---- shared expert stage 1, chunked; the router and its selection
        # chains are interleaved between chunks so the PE transposes they
        # need never head-block the matmul stream.
        actT = pact.tile([P, NHS, TOK], BF16, tag="actT")

        def shared_chunk(hc):
            csl = slice(hc * HC, (hc + 1) * HC)
            if hc in preload:
                w1c, w3c = preload[hc]
            else:
                w1c = pstream.tile([P, KD, HC], BF16, tag="wstream")
                nc.sync.dma_start(w1c[:], sw1T_v[:, :, csl])
                w3c = pstream.tile([P, KD, HC], BF16, tag="wstream")
                nc.sync.dma_start(w3c[:], sw3T_v[:, :, csl])
            for ht in range(HTPC):
                hsl = slice(ht * P, (ht + 1) * P)
                hidx = hc * HTPC + ht
                ph1 = ps.tile([P, TOK], F32, tag="ps")
                for k in range(KD):
                    nc.tensor.matmul(ph1[:], w1c[:, k, hsl], xk[k][:],
                                     start=(k == 0), stop=(k == KD - 1))
                ph3 = ps.tile([P, TOK], F32, tag="ps")
                for k in range(KD):
                    nc.tensor.matmul(ph3[:], w3c[:, k, hsl], xk[k][:],
                                     start=(k == 0), stop=(k == KD - 1))
                tsil = ptmp.tile([P, TOK], F32, tag="tmp")
                nc.scalar.activation(tsil[:], ph1[:], AF.Silu)
                nc.vector.scalar_tensor_tensor(
                    actT[:, hidx, :], tsil[:], OUT_SCALE, ph3[:],
                    op0=ALU.mult, op1=ALU.mult)

        combT = prt.tile([E, TOK], BF16, tag="combT")
        logitT = prt.tile([E, TOK], F32, tag="logitT")
        combs = {}

        def router_mm():
            ps_sc = ps.tile([E, TOK], F32, tag="ps")
            for k in range(KD):
                nc.tensor.matmul(ps_sc[:], rwt[:, k, :], xf[k][:],
                                 start=(k == 0), stop=(k == KD - 1))
            ps_tb = ps.tile([E, 1], F32, tag="ps")
            for k in range(KD):
                nc.tensor.matmul(ps_tb[:], rtwt[:, k, :], tembt[:, k, :],
                                 start=(k == 0), stop=(k == KD - 1))
            tb_sb = prt.tile([E, 1], F32, tag="tb")
            nc.vector.tensor_copy(tb_sb[:], ps_tb[:])
            # selection in logit space: monotone in sigmoid(s), avoids
            # LUT-error top-k flips on near-ties
            nc.vector.tensor_tensor(logitT[:], ps_sc[:],
                                    tb_sb[:].to_broadcast([E, TOK]), ALU.add)

        def router_front(m):
            # logit transpose + the serial DVE/ACT selection chain; the comb
            # write-back transpose is deferred to router_back one chunk later.
            tsl = slice(m * P, (m + 1) * P)
            ps_t = ps.tile([P, E], F32, tag="ps")
            nc.tensor.matmul(ps_t[:], logitT[:, tsl], ident[:E, :E],
                             is_transpose=True, start=True, stop=True)
            l_tok = prt.tile([P, E], F32, tag=f"l_tok{m % 2}")
            nc.vector.tensor_copy(l_tok[:], ps_t[:])
            s_tok = prt.tile([P, E], F32, tag=f"s_tok{m % 2}")
            nc.scalar.activation(s_tok[:], l_tok[:], AF.Sigmoid)
            sel = prt.tile([P, E], F32, tag=f"sel{m % 2}")
            nc.vector.tensor_add(sel[:], l_tok[:], biasb[:])
            m8 = prt.tile([P, E], F32, tag=f"m8{m % 2}")
            nc.vector.max(m8[:], sel[:])
            mask = prt.tile([P, E], F32, tag=f"mask{m % 2}")
            nc.vector.tensor_tensor(mask[:], sel[:],
                                    m8[:, 1:2].to_broadcast([P, E]), ALU.is_ge)
            sm = prt.tile([P, E], F32, tag=f"sm{m % 2}")
            nc.vector.tensor_mul(sm[:], s_tok[:], mask[:])
            den = prt.tile([P, 1], F32, tag=f"den{m % 2}")
            nc.vector.tensor_reduce(den[:], sm[:], axis=AX.X, op=ALU.add)
            nc.vector.tensor_scalar_add(den[:], den[:], EPS)
            rec = prt.tile([P, 1], F32, tag=f"rec{m % 2}")
            nc.vector.reciprocal(rec[:], den[:])
            comb = prt.tile([P, E], F32, tag=f"comb{m % 2}")
            nc.vector.scalar_tensor_tensor(
                comb[:], sm[:], OUT_SCALE, rec[:].to_broadcast([P, E]),
                op0=ALU.mult, op1=ALU.mult)
            combs[m] = comb

        def router_back(m):
            tsl = slice(m * P, (m + 1) * P)
            ps_ct = ps.tile([E, P], F32, tag="ps")
            nc.tensor.matmul(ps_ct[:], combs[m][:], ident[:],
                             is_transpose=True, start=True, stop=True)
            nc.vector.tensor_copy(combT[:, tsl], ps_ct[:])

        shared_chunk(0)
        shared_chunk(1)
        router_mm()
        router_front(0)
        shared_chunk(2)
        for m in range(1, TOK // P):
            router_back(m - 1)
            router_front(m)
            shared_chunk(2 + m)
        router_back(3)
        # comb rows are complete now: stage the partition-0 copies the
        # broadcast matmuls need while the remaining shared chunks compute.
        crows = []
        for e in range(E):
            crow = pcb.tile([1, TOK], BF16, tag="crow")
            nc.sync.dma_start(crow[:], combT[e:e + 1, :])
            crows.append(crow)
        for hc in range(6, NHC):
            shared_chunk(hc)

        # ---- gate rows: broadcast comb[e, :] across the 128 partitions ----
        cbbs = []
        for e in range(E):
            ps_cb = ps.tile([P, TOK], F32, tag="ps")
            nc.tensor.matmul(ps_cb[:], ones1[:], crows[e][:],
                             start=True, stop=True)
            cbb = pcb.tile([P, TOK], BF16, tag="cbb")
            nc.vector.tensor_copy(cbb[:], ps_cb[:])
            cbbs.append(cbb)

        # ---- routed experts stage 1: H[e*4+ht] = gelu(rw1[e] @ x) * gate ----
        # (matmuls + gelu have no router dependence; the gate multiply uses
        # the prebuilt cbb rows, so DVE gating overlaps the matmul stream)
        H = pact.tile([P, E * NHR, TOK], BF16, tag="H")
        for e in range(E):
            r1c = pstream.tile([P, KD, HR], BF16, tag="wstream")
            nc.sync.dma_start(r1c[:], rw1T_v[:, e, :, :])
            for ht in range(NHR):
                hsl = slice(ht * P, (ht + 1) * P)
                ph = ps.tile([P, TOK], F32, tag="ps")
                for k in range(KD):
                    nc.tensor.matmul(ph[:], r1c[:, k, hsl], xk[k][:],
                                     start=(k == 0), stop=(k == KD - 1))
                tgel = ptmp.tile([P, TOK], BF16, tag="tgel")
                nc.scalar.activation(tgel[:], ph[:], AF.Gelu)
                nc.vector.tensor_mul(H[:, e * NHR + ht, :], tgel[:], cbbs[e][:])

        # ---- stage 2: out[dt] = sum_h sw2T actT + sum_e,k rw2T H ----
        for dt in range(ND):
            dsl = slice(dt * P, (dt + 1) * P)
            w2c = pw2.tile([P, NHS, P], BF16, tag="w2stream")
            nc.sync.dma_start(w2c[:], sw2T_v[:, :, dsl])
            r2c = pw2.tile([P, E * NHR, P], BF16, tag="w2stream")
            nc.sync.dma_start(r2c[:], rw2T_v[:, :, :, dsl].rearrange(
                "p e k d -> p (e k) d"))
            po = ps.tile([P, TOK], F32, tag="ps")
            nmm = NHS + E * NHR
            i = 0
            for k in range(NHS):
                nc.tensor.matmul(po[:], w2c[:, k, :], actT[:, k, :],
                                 start=(i == 0), stop=(i == nmm - 1))
                i += 1
            for k in range(E * NHR):
                nc.tensor.matmul(po[:], r2c[:, k, :], H[:, k, :],
                                 start=(i == 0), stop=(i == nmm - 1))
                i += 1
            ot = pout.tile([P, TOK], F32, tag="ot")
            nc.vector.tensor_copy(ot[:], po[:])
            nc.sync.dma_start(outT_v[dt], ot[:])


def _make_in_maps(inputs):
    x_flat = np.asarray(inputs["x"], np.float32).reshape(N, D)
    t_emb = np.asarray(inputs["t_emb"], np.float32)
    shared_in = {
        "rwT": np.ascontiguousarray(np.asarray(inputs["router_w"], np.float32).T),
        "rtwT": np.ascontiguousarray(np.asarray(inputs["router_t_w"], np.float32).T),
        "biasB": np.ascontiguousarray(np.broadcast_to(
            np.asarray(inputs["router_bias"], np.float32)[None, :], (P, E))),
        "sw1T": np.ascontiguousarray(
            np.asarray(inputs["sw1"], np.float32).T.astype(NB)),
        "sw3T": np.ascontiguousarray(
            np.asarray(inputs["sw3"], np.float32).T.astype(NB)),
        "sw2T": np.ascontiguousarray(
            np.asarray(inputs["sw2"], np.float32).T.astype(NB)),
        "rw1T": np.ascontiguousarray(
            np.asarray(inputs["rw1"], np.float32).transpose(0, 2, 1).astype(NB)),
        "rw2T": np.ascontiguousarray(
            np.asarray(inputs["rw2"], np.float32).transpose(0, 2, 1).astype(NB)),
    }
    in_maps = []
    for c in range(NCORES):
        sl = x_flat[c * TOK:(c + 1) * TOK]
        batch = (c * TOK) // T
        m = dict(shared_in)
        xt = np.ascontiguousarray(sl.T)
        m["xT"] = xt.astype(NB)
        m["xTf"] = xt
        m["tembT"] = np.ascontiguousarray(t_emb[batch].reshape(D, 1))
        in_maps.append(m)
    return in_maps


_NC_CACHE = None


def _get_nc(reps=1):
    global _NC_CACHE
    if _NC_CACHE is None:
        _NC_CACHE = {}
    if reps not in _NC_CACHE:
        nc = bacc.Bacc(None, target_bir_lowering=False)
        with tile.TileContext(nc) as tc:
            _emit(nc, tc, reps=reps)
        nc.finalize()
        _NC_CACHE[reps] = nc
    return _NC_CACHE[reps]


def kernel(x, t_emb, router_w, router_t_w, router_bias, sw1, sw3, sw2, rw1, rw2):
    nc = _get_nc()
    in_maps = _make_in_maps(dict(
        x=x, t_emb=t_emb, router_w=router_w, router_t_w=router_t_w,
        router_bias=router_bias, sw1=sw1, sw3=sw3, sw2=sw2, rw1=rw1, rw2=rw2))

    res = run_bass_kernel_spmd(nc, in_maps, list(range(NCORES)))
    outs = [res.results[c]["outT"] for c in range(NCORES)]
    out = np.concatenate([o.T for o in outs], axis=0)
    return np.ascontiguousarray(out.reshape(B, T, D).astype(np.float32))
